# revision 12
# baseline (speedup 1.0000x reference)
"""LowRankSNN Trainium2 kernel: 8-core SPMD theta-neuron network simulation.

Strategy:
  - Row-shard the 4000x4000 conn across 8 cores (500 rows each, padded to 512).
  - Per timestep: TensorE matvec (spikes stationary [128,2] E/I-masked, connT
    moving [128,512] fp32), PE transpose of the [2,512] drive into [128,8],
    short fused DVE chain for conductance/phase/spike updates, then a 7-way
    remote SBUF->SBUF DMA broadcast of the local spike slice (XOR slotting).
  - conn columns are XOR-permuted per core so every core's own slice sits at
    slot 0 and all cross-core addressing is compile-time constant.
  - Per-neuron G_P scaling is folded into conn rows on the host; E/I column
    split is done by masking the spike stationary, so dA/dB come out of one
    matmul pass.
"""

import numpy as np

# Model constants
V_T, V_R = -55.0, -62.0
REV_I = -70.0
G_P = (0.004069, 0.02672, 0.003276, 0.02138)
C0 = np.float32(2.0 / 7.0)
TAUD_E, TAUD_I = 2.0, 5.0
DEC_E, DEC_I = 0.5, 0.8  # 1 - dt/tau for dt=1
N_E, N_I = 3200, 800
N = 4000
NP = 4096          # padded neuron count
RP = 512           # padded rows per core
RR = 500           # real rows per core
NCORE = 8
PI = float(np.pi)
f32n = np.float32

_BUILD_CACHE = {}


def _prep_core_inputs(Input, conn, T):
    """Host-side prep. Returns in_maps (list of 8 dicts) for the device."""
    in_maps = []
    for r in range(NCORE):
        rows = np.arange(RR) + RR * 0 + 500 * r
        rowE = rows < N_E
        kA = np.where(rowE, f32n(G_P[0]), f32n(G_P[2]))
        kB = np.where(rowE, f32n(G_P[1]), f32n(G_P[3]))
        connT_s = np.zeros((NP, RP), np.float32)
        for s in range(NCORE):
            c = r ^ s
            cols = np.arange(RR) + 500 * c
            colE = cols < N_E
            blk = conn[np.ix_(rows, cols)].astype(np.float32)
            scale = np.where(colE[None, :], kA[:, None], kB[:, None])
            connT_s[512 * s:512 * s + RR, :RR] = (blk * scale).T
        # device layout: [128, 32*512], col 512c+j <- connT_s[128c+p, j]
        connT_dev = np.ascontiguousarray(
            connT_s.reshape(32, 128, RP).transpose(1, 0, 2).reshape(128, 32 * RP))

        inp_l = np.zeros((RP, T), np.float32)
        inp_l[:RR] = Input[rows]
        inp_dev = np.ascontiguousarray(
            inp_l.reshape(4, 128, T).transpose(1, 0, 2).reshape(128, 4 * T))

        gL = np.zeros(RP, np.float32)
        gL[:RR] = np.where(rowE, f32n(0.08), f32n(0.1))
        maskE = np.zeros(RP, np.float32)
        maskI = np.zeros(RP, np.float32)
        maskE[:RR] = rowE.astype(np.float32)
        maskI[:RR] = (~rowE).astype(np.float32)
        consts = np.zeros((128, 16), np.float32)
        consts[:, 0:4] = gL.reshape(4, 128).T
        consts[:, 4:8] = maskE.reshape(4, 128).T
        consts[:, 8:12] = maskI.reshape(4, 128).T
        consts[0, 12] = 1.0  # identity[0,0]
        consts[1, 13] = 1.0  # identity[1,1]
        consts[:, 14] = np.float32(np.pi / 2)  # bias vector for cos-via-sin
        in_maps.append({"connT": connT_dev, "inp": inp_dev, "consts": consts})
    return in_maps


def _build_nc(T, d2d_xor=2, races=False, T_sim=None):
    """Build the SPMD device program for T timesteps (steps t=1..T-1).

    T_sim (default T): number of simulated steps is T_sim-1; I/O shapes stay
    keyed to T so differential timing isolates pure device step cost."""
    import concourse.bacc as bacc
    import concourse.mybir as mybir

    f32 = mybir.dt.float32
    ALU = mybir.AluOpType
    ACT_SIN = mybir.ActivationFunctionType.Sin

    if T_sim is None:
        T_sim = T
    nc = bacc.Bacc("TRN2", num_devices=NCORE, debug=False,
                   detect_race_conditions=races)

    connT_d = nc.dram_tensor("connT", [128, 32 * 512], f32, kind="ExternalInput")
    inp_d = nc.dram_tensor("inp", [128, 4 * T], f32, kind="ExternalInput")
    consts_d = nc.dram_tensor("consts", [128, 16], f32, kind="ExternalInput")
    V_d = nc.dram_tensor("V", [128, 4 * T], f32, kind="ExternalOutput")
    PH_d = nc.dram_tensor("PH", [128, 4 * T], f32, kind="ExternalOutput")
    SP_d = nc.dram_tensor("SP", [128, 4 * T], f32, kind="ExternalOutput")
    IS_d = nc.dram_tensor("IS", [128, 4 * T], f32, kind="ExternalOutput")

    connT = nc.alloc_sbuf_tensor("connT_sb", [128, 32 * 512], f32)
    inp = nc.alloc_sbuf_tensor("inp_sb", [128, 4 * T], f32)
    consts = nc.alloc_sbuf_tensor("consts_sb", [128, 16], f32)
    Vb = nc.alloc_sbuf_tensor("Vb", [128, 4 * T], f32)
    PHb = nc.alloc_sbuf_tensor("PHb", [128, 4 * T], f32)
    SPb = nc.alloc_sbuf_tensor("SPb", [128, 4 * T], f32)
    ISb = nc.alloc_sbuf_tensor("ISb", [128, 4 * T], f32)
    stat = [nc.alloc_sbuf_tensor(f"stat{i}", [128, 64], f32) for i in range(3)]
    d_sb = [nc.alloc_sbuf_tensor(f"d_sb{i}", [2, 512], f32) for i in range(2)]
    g2 = nc.alloc_sbuf_tensor("g2", [128, 8], f32)
    absb = nc.alloc_sbuf_tensor("absb", [128, 4], f32)
    s2b = nc.alloc_sbuf_tensor("s2b", [128, 4], f32)
    c2b = nc.alloc_sbuf_tensor("c2b", [128, 4], f32)
    cospb = nc.alloc_sbuf_tensor("cospb", [128, 4], f32)
    rc2b = nc.alloc_sbuf_tensor("rc2b", [128, 4], f32)
    tanb = nc.alloc_sbuf_tensor("tanb", [128, 4], f32)
    B1b = nc.alloc_sbuf_tensor("B1b", [128, 4], f32)
    wb = nc.alloc_sbuf_tensor("wb", [128, 4], f32)
    gLcb = nc.alloc_sbuf_tensor("gLcb", [128, 4], f32)
    pmb = nc.alloc_sbuf_tensor("pmb", [128, 4], f32)
    P1b = nc.alloc_sbuf_tensor("P1b", [128, 4], f32)
    gsb = nc.alloc_sbuf_tensor("gsb", [128, 4], f32)
    m1b = nc.alloc_sbuf_tensor("m1b", [128, 4], f32)
    tmpb = nc.alloc_sbuf_tensor("tmpb", [128, 4], f32)
    ph1b = nc.alloc_sbuf_tensor("ph1b", [128, 4], f32)

    psum_mv = [nc.alloc_psum_tensor(f"pmv{i}", [2, 512], f32) for i in range(2)]
    psum_d = [nc.alloc_psum_tensor(f"pd{i}", [128, 8], f32) for i in range(2)]

    in_sem = nc.alloc_semaphore("in_sem")
    init_sem = nc.alloc_semaphore("init_sem")
    mm_sem = nc.alloc_semaphore("mm_sem")
    dcp_sem = nc.alloc_semaphore("dcp_sem")
    tr_sem = nc.alloc_semaphore("tr_sem")
    ch_sem = nc.alloc_semaphore("ch_sem")
    sE_sem = nc.alloc_semaphore("sE_sem")
    tail_sem = nc.alloc_semaphore("tail_sem")
    act_sem = nc.alloc_semaphore("act_sem")
    prep_sem = nc.alloc_semaphore("prep_sem")
    lsem = nc.alloc_semaphore("lsem")
    fin_sem = nc.alloc_semaphore("fin_sem")
    out_sem = nc.alloc_semaphore("out_sem")
    slot_sems = {j: nc.alloc_semaphore(f"slot_sem_{j}") for j in range(1, 8)}

    # per-step [128,4] views of the time-indexed buffers
    def col(buf, t):
        return buf[:, t:t + 3 * T + 1:T]

    ident = consts[0:2, 12:14]
    gl_v = consts[:, 0:4]
    mE_v = consts[:, 4:8]
    mI_v = consts[:, 8:12]
    hpi_v = consts[:, 14:15]

    with nc.Block() as blk:

        @blk.sync
        def _(sync):
            sync.dma_start(connT[:, :], connT_d[:, :]).then_inc(in_sem, 16)
            sync.dma_start(inp[:, :], inp_d[:, :]).then_inc(in_sem, 16)
            sync.dma_start(consts[:, :], consts_d[:, :]).then_inc(in_sem, 16)
            sync.wait_ge(tail_sem, T_sim - 1)
            sync.wait_ge(fin_sem, 1)
            sync.dma_start(V_d[:, :], Vb[:, :]).then_inc(out_sem, 16)
            sync.dma_start(PH_d[:, :], PHb[:, :]).then_inc(out_sem, 16)
            sync.dma_start(SP_d[:, :], SPb[:, :]).then_inc(out_sem, 16)
            sync.dma_start(IS_d[:, :], ISb[:, :]).then_inc(out_sem, 16)
            sync.wait_ge(out_sem, 64)

        @blk.vector
        def _(vec):
            vec.wait_ge(in_sem, 48)
            vec.memset(stat[0][:, :], 0.0)
            vec.memset(g2[:, :], 0.0)
            vec.memset(col(PHb, 0), 0.0)
            vec.memset(col(Vb, T - 1), 0.0)
            vec.memset(col(SPb, 0), 0.0)
            vec.memset(col(ISb, 0), 0.0)
            vec.drain().then_inc(init_sem, 1)
            for t in range(1, T_sim):
                b = t % 2
                pc = t % 3
                pp = (t - 1) % 3
                # ---- off-path (runs during this step's matvec) ----
                if t >= 2:
                    # spike output column for t-1 from slot-0 of prev stat buf
                    vec.tensor_tensor(col(SPb, t - 1), stat[pp][:, 0:8:2],
                                      stat[pp][:, 1:8:2], ALU.add)
                vec.wait_ge(act_sem, 3 * (t - 1) + 2)
                vec.reciprocal(rc2b[:, :], c2b[:, :])
                vec.drain()
                vec.tensor_tensor(tanb[:, :], s2b[:, :], rc2b[:, :], ALU.mult)
                vec.drain()
                vec.tensor_scalar(col(Vb, t - 1), tanb[:, :], 3.5, -58.5,
                                  ALU.mult, ALU.add)
                vec.wait_ge(act_sem, 3 * t)
                vec.tensor_scalar(B1b[:, :], cospb[:, :], 1.0, float(C0),
                                  ALU.add, ALU.mult)
                vec.tensor_tensor(gLcb[:, :], gl_v, cospb[:, :], ALU.mult)
                vec.drain()
                vec.tensor_tensor(wb[:, :], B1b[:, :], col(inp, t), ALU.mult)
                vec.tensor_tensor(pmb[:, :], col(PHb, t - 1), gLcb[:, :],
                                  ALU.subtract)
                vec.drain()
                vec.tensor_tensor(P1b[:, :], pmb[:, :], wb[:, :], ALU.add)
                # ---- psum -> sbuf copy for the transpose ----
                vec.wait_ge(mm_sem, t)
                vec.tensor_copy(d_sb[b][:, :], psum_mv[b][:, :]).then_inc(dcp_sem, 1)
                # ---- tail (critical path) ----
                vec.wait_ge(tr_sem, t)
                vec.scalar_tensor_tensor(g2[:, 0:8:2], g2[:, 0:8:2], DEC_E,
                                         psum_d[b][:, 0:8:2], ALU.mult, ALU.add)
                vec.scalar_tensor_tensor(g2[:, 1:8:2], g2[:, 1:8:2], DEC_I,
                                         psum_d[b][:, 1:8:2], ALU.mult,
                                         ALU.add).then_inc(ch_sem, 1)
                vec.drain()
                vec.tensor_tensor(gsb[:, :], g2[:, 0:8:2], g2[:, 1:8:2], ALU.add)
                vec.drain()
                vec.tensor_tensor(m1b[:, :], gsb[:, :], col(Vb, t - 1), ALU.mult)
                vec.drain()
                vec.scalar_tensor_tensor(col(ISb, t), g2[:, 1:8:2], -70.0,
                                         m1b[:, :], ALU.mult, ALU.subtract)
                vec.drain()
                vec.tensor_tensor(tmpb[:, :], B1b[:, :], col(ISb, t), ALU.mult)
                vec.drain()
                vec.tensor_tensor(ph1b[:, :], P1b[:, :], tmpb[:, :], ALU.add)
                vec.drain()
                if t > 3:
                    vec.wait_ge(lsem, 112 * (t - 3))
                vec.scalar_tensor_tensor(stat[pc][:, 0:8:2], ph1b[:, :], PI,
                                         mE_v, ALU.is_ge, ALU.mult)
                vec.scalar_tensor_tensor(stat[pc][:, 1:8:2], ph1b[:, :], PI,
                                         mI_v, ALU.is_ge,
                                         ALU.mult).then_inc(sE_sem, 1)
                vec.drain()
                vec.scalar_tensor_tensor(ph1b[:, :], stat[pc][:, 0:8:2],
                                         -2.0 * PI, ph1b[:, :], ALU.mult, ALU.add)
                vec.drain()
                vec.scalar_tensor_tensor(col(PHb, t), stat[pc][:, 1:8:2],
                                         -2.0 * PI, ph1b[:, :], ALU.mult,
                                         ALU.add).then_inc(tail_sem, 1)
            # final spike column
            pl = (T_sim - 1) % 3
            vec.tensor_tensor(col(SPb, T_sim - 1), stat[pl][:, 0:8:2],
                              stat[pl][:, 1:8:2], ALU.add).then_inc(fin_sem, 1)

        @blk.scalar
        def _(act):
            ACT_ABS = mybir.ActivationFunctionType.Abs
            act.wait_ge(init_sem, 1)
            for t in range(1, T_sim):
                if t >= 2:
                    act.wait_ge(tail_sem, t - 1)
                ph_prev = col(PHb, t - 1)
                # range-reduced: sin args must stay in [-pi, pi] on ACT
                act.activation(absb[:, :], ph_prev, ACT_ABS)
                act.activation(s2b[:, :], ph_prev, ACT_SIN,
                               scale=0.5).then_inc(act_sem, 1)
                act.drain()
                # cos(x) = sin(pi/2 - |x|); cos(x/2) = sin(pi/2 - |x|/2)
                act.activation(c2b[:, :], absb[:, :], ACT_SIN, bias=hpi_v,
                               scale=-0.5).then_inc(act_sem, 1)
                act.activation(cospb[:, :], absb[:, :], ACT_SIN, bias=hpi_v,
                               scale=-1.0).then_inc(act_sem, 1)

        @blk.tensor
        def _(te):
            te.wait_ge(in_sem, 48)
            te.wait_ge(init_sem, 1)
            for t in range(1, T_sim):
                b = t % 2
                pp = (t - 1) % 3
                mm = None
                for c in range(32):
                    s, q = divmod(c, 4)
                    if q == 0:
                        if s == 0:
                            te.wait_ge(sE_sem, t - 1)
                        else:
                            te.wait_ge(slot_sems[s], 2 * (t - 1))
                    mm = nc.tensor.matmul(
                        psum_mv[b][:, :],
                        stat[pp][:, 8 * s + 2 * q:8 * s + 2 * q + 2],
                        connT[:, 512 * c:512 * (c + 1)],
                        start=(c == 0), stop=(c == 31))
                mm.then_inc(mm_sem, 1)
                te.wait_ge(dcp_sem, t)
                if t > 2:
                    te.wait_ge(ch_sem, t - 2)
                tr = None
                for g in range(4):
                    tr = nc.tensor.transpose(
                        psum_d[b][:, 2 * g:2 * g + 2],
                        d_sb[b][:, 128 * g:128 * (g + 1)], ident)
                tr.then_inc(tr_sem, 1)

        @blk.gpsimd
        def _(gp):
            for t in range(1, T_sim):
                pc = t % 3
                for j in range(1, 8):
                    jj = j if j < 4 else j ^ d2d_xor
                    rdests = [(0, jj) if k == j else None for k in range(8)]
                    gp.remote_dma_broadcast(
                        out_ap=stat[pc][:, 8 * j:8 * j + 8],
                        in_ap=stat[pc][:, 0:8],
                        remote_sem=slot_sems[j],
                        local_sem=lsem,
                        rdests=rdests).then_inc(prep_sem, 1)
                gp.wait_ge(prep_sem, 7 * t)
                gp.wait_ge(sE_sem, t)
                gp.trigger_dma(count=7)

    nc.compile()
    return nc


def _get_nc(T, d2d_xor=2, races=False, T_sim=None):
    key = (T, d2d_xor, races, T_sim)
    if key not in _BUILD_CACHE:
        _BUILD_CACHE[key] = _build_nc(T, d2d_xor, races, T_sim=T_sim)
    return _BUILD_CACHE[key]


def _decode_outputs(results, T, W_out):
    def gather(name):
        out = np.zeros((N, T), np.float32)
        for r in range(NCORE):
            dev = results[r][name]  # [128, 4T]
            loc = dev.reshape(128, 4, T).transpose(1, 0, 2).reshape(RP, T)
            out[500 * r:500 * r + RR] = loc[:RR]
        return out
    V = gather("V")
    PH = gather("PH")
    SP = gather("SP")
    IS = gather("IS")
    readout = (W_out.astype(np.float32) @ SP).astype(np.float32)
    return V, PH, SP, IS, readout


def _numpy_fallback(dt, Input, conn, W_out):
    """Reference-faithful numpy simulation (slow path for unexpected inputs)."""
    dtf = np.float32
    Input = np.asarray(Input, np.float32)
    conn = np.asarray(conn, np.float32)
    W_out = np.asarray(W_out, np.float32)
    n = Input.shape[0]
    T = Input.shape[1]
    ne = int(round(n * 0.8))
    dt = dtf(dt)
    cEE = conn[:ne, :ne]; cEI = conn[:ne, ne:]
    cIE = conn[ne:, :ne]; cII = conn[ne:, ne:]
    gL = np.concatenate([np.full(ne, 0.08), np.full(n - ne, 0.1)]).astype(dtf)
    phase = np.zeros(n, dtf); spk = np.zeros(n, dtf)
    gEE = np.zeros(ne, dtf); gEI = np.zeros(ne, dtf)
    gIE = np.zeros(n - ne, dtf); gII = np.zeros(n - ne, dtf)
    V = np.zeros((n, T), dtf); PH = np.zeros((n, T), dtf)
    SP = np.zeros((n, T), dtf); IS = np.zeros((n, T), dtf)
    pi = dtf(np.pi)
    for t in range(1, T):
        inp = Input[:, t]
        sE, sI = spk[:ne], spk[ne:]
        dEE = cEE @ sE; dEI = cEI @ sI
        dIE = cIE @ sE; dII = cII @ sI
        gEE = gEE + dt * (-gEE / dtf(TAUD_E) + dtf(G_P[0]) * dEE / dt)
        gEI = gEI + dt * (-gEI / dtf(TAUD_I) + dtf(G_P[1]) * dEI / dt)
        gIE = gIE + dt * (-gIE / dtf(TAUD_E) + dtf(G_P[2]) * dIE / dt)
        gII = gII + dt * (-gII / dtf(TAUD_I) + dtf(G_P[3]) * dII / dt)
        Vp = dtf((V_T + V_R) / 2.0) + dtf((V_T - V_R) / 2.0) * np.tan(phase / dtf(2.0))
        IsE = -gEE * Vp[:ne] - gEI * (Vp[:ne] - dtf(REV_I))
        IsI = -gIE * Vp[ne:] - gII * (Vp[ne:] - dtf(REV_I))
        Is = np.concatenate([IsE, IsI])
        cosp = np.cos(phase)
        phase_new = phase + dt * (-gL * cosp + dtf(2.0 / 7.0) * (dtf(1) + cosp) * (Is + inp))
        spk = (phase_new >= pi).astype(dtf)
        phase_new = phase_new - dtf(2.0) * pi * spk
        V[:, t - 1] = Vp
        PH[:, t] = phase_new
        SP[:, t] = spk
        IS[:, t] = Is
        phase = phase_new
    readout = W_out @ SP
    return V, PH, SP, IS, readout


def kernel(dt, Input, conn, W_out):
    Input = np.asarray(Input)
    conn = np.asarray(conn)
    W_out = np.asarray(W_out)
    dt_val = int(np.asarray(dt).reshape(-1)[0]) if np.asarray(dt).size else 1
    T = Input.shape[1]
    if (dt_val != 1 or Input.shape[0] != N or conn.shape != (N, N)):
        return _numpy_fallback(dt, Input, conn, W_out)

    from concourse import bass_utils
    nc = _get_nc(T)
    in_maps = _prep_core_inputs(Input.astype(np.float32),
                                conn.astype(np.float32), T)
    res = bass_utils.run_bass_kernel_spmd(nc, in_maps,
                                          core_ids=list(range(NCORE)))
    return _decode_outputs(res.results, T, W_out)

